# revision 11
# baseline (speedup 1.0000x reference)
"""AUGRU Trainium2 kernel — 2-chain pipelined, parameterized emission order.

See kernel_v2.py docstring for the design rationale.  This variant factors
the per-(step, chain) work into stage closures and supports several
emission templates (EMIT_MODE) to control each in-order engine queue:

  1: stage-interleaved   [mmA mmB | sA sB | dA dB | uA uB | tA tB | eA eB]
  2: body-interleaved    [A: mm s d u t e][B: mm s d u t e]
  3: half-step shifted   per t: [A-half1(t), B-half2(t-1), A-half2(t),
                                 B-half1(t)]
"""

import sys
import numpy as np
import ml_dtypes

for _p in ("/opt/trn_rl_repo",):
    if _p not in sys.path:
        sys.path.insert(0, _p)

import concourse.bacc as bacc
import concourse.mybir as mybir
import concourse.tile as tile
from concourse.bass_utils import run_bass_kernel_spmd

B, T, I, H = 1024, 200, 128, 128
NCORES = 8
BL = B // NCORES   # 128 batch rows per core
BC = BL // 2       # 64 batch rows per chain
C = 4              # t-steps per x-projection group
NG = T // C        # 50 groups
CH = 20            # t-steps per attention DMA chunk
F32 = mybir.dt.float32
BF16 = mybir.dt.bfloat16
AF = mybir.ActivationFunctionType
ALU = mybir.AluOpType
BF16NP = ml_dtypes.bfloat16

EMIT_MODE = 3

_compiled = None


def build_program():
    nc = bacc.Bacc("TRN2", target_bir_lowering=False, debug=False,
                   num_devices=NCORES)

    xT = nc.dram_tensor("xT", [T, I, BL], BF16, kind="ExternalInput").ap()
    aT = nc.dram_tensor("aT", [128, T * BL], BF16, kind="ExternalInput").ap()
    wcat_in = nc.dram_tensor("wcat", [128, 8 * 128], BF16,
                             kind="ExternalInput").ap()
    brow_in = nc.dram_tensor("brow", [1, 2 * 128], BF16,
                             kind="ExternalInput").ap()
    ones_in = nc.dram_tensor("ones", [1, C * BC], BF16,
                             kind="ExternalInput").ap()
    bh_in = nc.dram_tensor("bh", [128, 1], F32, kind="ExternalInput").ap()
    h_out = nc.dram_tensor("h_out", [H, BL], F32, kind="ExternalOutput").ap()

    with tile.TileContext(nc) as tc:
        with (
            tc.tile_pool(name="weights", bufs=1) as wpool,
            tc.tile_pool(name="xin", bufs=5) as xpool,
            tc.tile_pool(name="abc", bufs=2) as abcpool,
            tc.tile_pool(name="state", bufs=4) as spool,
            tc.tile_pool(name="ps", bufs=2, space="PSUM") as pspool,
        ):
            wcat = wpool.tile([128, 8 * 128], BF16, name="wcat", tag="wcat")
            nc.sync.dma_start(wcat[:], wcat_in)
            brow = wpool.tile([1, 2 * 128], BF16, name="brow", tag="brow")
            nc.sync.dma_start(brow[:], brow_in)
            ones = wpool.tile([1, C * BC], BF16, name="ones", tag="ones")
            nc.sync.dma_start(ones[:], ones_in)
            bh = wpool.tile([128, 1], F32, name="bh", tag="bh")
            nc.sync.dma_start(bh[:], bh_in)
            wnames = ("wrx", "wzx", "whx", "ur", "uz", "uh", "urn", "uzn")
            w = {n: wcat[:, k * 128:(k + 1) * 128]
                 for k, n in enumerate(wnames)}

            warmup = spool.tile([128, 1], BF16, name="warmup", tag="warmup",
                                bufs=1)
            nc.scalar.activation(warmup[:], bh[:], AF.Sigmoid)

            GW = 2 * C * BC      # 512 psum cols per [r|z] tile
            h = [None, None]     # per-chain materialized h_{t-1}
            hparts = [[], []]    # per-chain [(tile, negated)] summing to h
            xg_t = {}
            psrz_t = {}
            psh_t = {}
            abc = {}             # chunk index -> attention tile
            rz_t, zp_t, t1n_t, rh_t, ht_t = {}, {}, {}, {}, {}

            def load_x(g):
                for X in (0, 1):
                    xg = xpool.tile([I, C * BC], BF16, name=f"xg{X}",
                                    tag=f"xg{X}")[:]
                    nc.sync.dma_start(
                        xg.rearrange("i (c b) -> i c b", c=C),
                        xT[g * C:(g + 1) * C, :,
                           X * BC:(X + 1) * BC].rearrange("c i b -> i c b"))
                    xg_t[(g, X)] = xg

            def load_abc(k):
                a = abcpool.tile([128, CH * BL], BF16, name="abc",
                                 tag="abc")[:]
                nc.sync.dma_start(a, aT[:, k * CH * BL:(k + 1) * CH * BL])
                abc[k] = a

            def alloc_ps(g):
                # one full PSUM bank per tile: [r|z] per chain, h per chain
                # (separate h tiles per chain so the tile-granularity
                # write-after-read tracking never couples the two chains'
                # Uh-matmul/tanh cycles; 4 tags x bufs=2 = all 8 banks)
                psrz_t[(g, 0)] = pspool.tile([128, GW], F32, name="psrzA",
                                             tag="psrzA")[:]
                psrz_t[(g, 1)] = pspool.tile([128, GW], F32, name="psrzB",
                                             tag="psrzB")[:]
                psh_t[(g, 0)] = pspool.tile([128, GW], F32, name="pshA",
                                            tag="pshA")[:]
                psh_t[(g, 1)] = pspool.tile([128, GW], F32, name="pshB",
                                            tag="pshB")[:]

            # all boundary matmuls are emitted in 128-col pieces: the tile
            # scheduler places them from its (inaccurate) cost model, and a
            # misplaced 600ns matmul head-of-line-blocks the critical
            # recurrent matmuls, while a 128-col piece costs <=150ns.
            HB = C * BC // 2    # 128 cols = half a gate's group block

            def emit_xproj_rz(g, X, piece):
                ps = psrz_t[(g, X)]
                lo, hi = piece * HB, (piece + 1) * HB
                nc.tensor.matmul(ps[:, lo:hi], w["wrx"],
                                 xg_t[(g, X)][:, lo:hi],
                                 start=(piece == 0), stop=False)
                nc.tensor.matmul(ps[:, C * BC + lo:C * BC + hi], w["wzx"],
                                 xg_t[(g, X)][:, lo:hi],
                                 start=False, stop=False)

            def emit_bias_rz(g, X, piece):
                ps = psrz_t[(g, X)]
                lo, hi = piece * HB, (piece + 1) * HB
                nc.tensor.matmul(ps[:, lo:hi], brow[:, 0:128],
                                 ones[:, 0:HB], start=False, stop=False)
                nc.tensor.matmul(ps[:, C * BC + lo:C * BC + hi],
                                 brow[:, 128:256], ones[:, 0:HB],
                                 start=False, stop=False)

            def emit_xproj_h(g, X, piece):
                lo, hi = piece * HB, (piece + 1) * HB
                nc.tensor.matmul(psh_t[(g, X)][:, lo:hi],
                                 w["whx"], xg_t[(g, X)][:, lo:hi],
                                 start=(piece == 0), stop=False)

            # ---- per-(step, chain) stages -------------------------------
            # The two h-parts (t1n, t2) become ready at different times:
            # t1n right after zp (early in step t-1), t2 only after tanh.
            # Emitting the t1n matmuls separately lets them execute during
            # tanh (PE otherwise idle), so sigma(t) waits only on the two
            # t2 matmuls.
            def st_mms_early(t, X):
                # urn/uzn @ t1n(t-1); t1n is ready long before tanh(t-1)
                # completes, so these run on PE during tanh.
                if t >= T or (t - 1, X) not in t1n_t:
                    return
                g, j = divmod(t, C)
                ps = psrz_t[(g, X)]
                hp = t1n_t[(t - 1, X)]
                nc.tensor.matmul(ps[:, j * BC:(j + 1) * BC], w["urn"], hp[:],
                                 start=False, stop=False)
                nc.tensor.matmul(ps[:, C * BC + j * BC:C * BC + (j + 1) * BC],
                                 w["uzn"], hp[:], start=False, stop=False)

            def st_mms_late(t, X):
                # ur/uz @ t2(t-1): the last accumulation sigma(t) waits on
                if t >= T:
                    return
                g, j = divmod(t, C)
                ps = psrz_t[(g, X)]
                for hp, neg in hparts[X]:
                    if not neg:
                        nc.tensor.matmul(ps[:, j * BC:(j + 1) * BC],
                                         w["ur"], hp[:],
                                         start=False, stop=False)
                        nc.tensor.matmul(
                            ps[:, C * BC + j * BC:C * BC + (j + 1) * BC],
                            w["uz"], hp[:], start=False, stop=True)

            def st_sigma(t, X):
                g, j = divmod(t, C)
                rz = spool.tile([H, 2 * BC], BF16, name=f"rz{X}",
                                tag=f"rz{X}")
                ps_view = psrz_t[(g, X)].rearrange(
                    "p (g c) -> p g c", g=2)[:, :, j * BC:(j + 1) * BC]
                nc.scalar.activation(
                    rz[:].rearrange("p (g c) -> p g c", g=2),
                    ps_view, AF.Sigmoid)
                rz_t[(t, X)] = rz

            def st_dve1(t, X):
                rz = rz_t[(t, X)]
                if h[X] is not None:
                    rh = spool.tile([H, BC], BF16, name=f"rh{X}",
                                    tag=f"rh{X}")
                    nc.vector.tensor_mul(rh[:], rz[:, 0:BC], h[X][:])
                    rh_t[(t, X)] = rh
                av = abc[t // CH][:, (t % CH) * BL + X * BC:
                                 (t % CH) * BL + (X + 1) * BC]
                zp = spool.tile([H, BC], BF16, name=f"zp{X}", tag=f"zp{X}")
                nc.vector.tensor_mul(zp[:], rz[:, BC:2 * BC], av)
                zp_t[(t, X)] = zp
                if h[X] is not None:
                    t1n = spool.tile([H, BC], BF16, name=f"t1n{X}",
                                     tag=f"t1n{X}")
                    nc.vector.scalar_tensor_tensor(
                        t1n[:], zp[:], 1.0, h[X][:],
                        ALU.subtract, ALU.mult)
                    t1n_t[(t, X)] = t1n

            def st_uh(t, X):
                g, j = divmod(t, C)
                if (t, X) in rh_t:
                    hsl = slice(j * BC, (j + 1) * BC)
                    nc.tensor.matmul(psh_t[(g, X)][:, hsl], w["uh"],
                                     rh_t[(t, X)][:], start=False, stop=True)

            def st_tanh(t, X):
                g, j = divmod(t, C)
                hsl = slice(j * BC, (j + 1) * BC)
                ht = spool.tile([H, BC], BF16, name=f"ht{X}", tag=f"ht{X}")
                nc.scalar.activation(ht[:], psh_t[(g, X)][:, hsl], AF.Tanh,
                                     bias=bh[:])
                ht_t[(t, X)] = ht

            def st_dve2(t, X):
                t2 = spool.tile([H, BC], BF16, name=f"t2{X}", tag=f"t2{X}")
                nc.vector.tensor_mul(t2[:], zp_t[(t, X)][:],
                                     ht_t[(t, X)][:])
                if h[X] is None:
                    hparts[X] = [(t2, False)]
                    h[X] = t2
                else:
                    t1n = t1n_t[(t, X)]
                    hparts[X] = [(t1n, True), (t2, False)]
                    if t == T - 1:
                        hn = spool.tile([H, BC], F32, name=f"hf{X}",
                                        tag=f"hf{X}", bufs=1)
                    else:
                        hn = spool.tile([H, BC], BF16, name=f"h{X}",
                                        tag=f"h{X}")
                    nc.vector.tensor_sub(hn[:], t2[:], t1n[:])
                    h[X] = hn

            def st_groupwork(t):
                g, j = divmod(t, C)
                if j == 0 and g + 2 < NG:
                    load_x(g + 2)
                if t % CH == 0 and t + CH < T:
                    load_abc(t // CH + 1)
                if j == 0 and g + 1 < NG:
                    alloc_ps(g + 1)

            # all of group g+1's x-proj/bias matmuls become data-ready at
            # the start of group g (x prefetched 2 groups out, PSUM bank
            # released), so the greedy scheduler front-loads all ~10 of
            # them into one PE blob that head-of-line-blocks the critical
            # recurrent matmuls.  bass_wait_until_ts (a scheduling-pass
            # hold, no HW effect) spreads them across the group's steps.
            P_EST_NS = 1850.0

            def st_grouppe(t):
                g, j = divmod(t, C)
                if g + 1 >= NG or j != 1:
                    return

                def hold(t_eff):
                    return tc.tile_wait_until(t_eff * P_EST_NS * 1e-6)

                # all of group g+1's boundary matmuls (4 rz calls = 8 mm,
                # 4 h calls = 4 mm, 4 bias calls = 8 mm) spread evenly
                # across the group span so no single step eats a PE blob
                calls = [lambda X=X, p=p: emit_xproj_rz(g + 1, X, p)
                         for X in (0, 1) for p in (0, 1)]
                calls += [lambda X=X, p=p: emit_xproj_h(g + 1, X, p)
                          for X in (0, 1) for p in (0, 1)]
                calls += [lambda X=X, p=p: emit_bias_rz(g + 1, X, p)
                          for X in (0, 1) for p in (0, 1)]
                for k, call in enumerate(calls):
                    with hold(g * C + 0.6 + (3.3 - 0.6) * k / 11.0):
                        call()

            # ---- emission templates -------------------------------------
            load_x(0)
            load_x(1)
            load_abc(0)
            alloc_ps(0)
            for X in (0, 1):
                for piece in (0, 1):
                    emit_xproj_rz(0, X, piece)
                    emit_bias_rz(0, X, piece)
                    emit_xproj_h(0, X, piece)

            # Stage-interleaved emission.  PE queue order per period matches
            # data-readiness order: uhB, mmE_B(t), uhA, mmL_B(t) [gates
            # sigmaB], mmE_A(t+1), mmL_A(t+1) [gates sigmaA(t+1)].
            # Scalar order: sigmaA, tanhB, tanhA, sigmaB.
            for t in range(T):
                st_groupwork(t)
                st_sigma(t, 0)
                st_dve1(t, 0)
                st_grouppe(t)
                if t > 0:
                    st_uh(t - 1, 1)
                    st_mms_early(t, 1)
                    st_tanh(t - 1, 1)
                    st_dve2(t - 1, 1)
                st_uh(t, 0)
                if t > 0:
                    st_mms_late(t, 1)
                st_mms_early(t + 1, 0)
                st_tanh(t, 0)
                st_dve2(t, 0)
                st_mms_late(t + 1, 0)
                st_sigma(t, 1)
                st_dve1(t, 1)
            st_uh(T - 1, 1)
            st_tanh(T - 1, 1)
            st_dve2(T - 1, 1)

            nc.sync.dma_start(h_out[:, 0:BC], h[0][:])
            nc.sync.dma_start(h_out[:, BC:BL], h[1][:])
    nc.compile()
    return nc


def _prep_inputs(inputs, attention_scores, Wz, bz, Wr, br, Wh, bh):
    """Shard + lay out host-side.  Returns per-core input maps."""
    x = np.asarray(inputs, dtype=np.float32)
    a = np.asarray(attention_scores, dtype=np.float32)
    Wz = np.asarray(Wz, dtype=np.float32)
    Wr = np.asarray(Wr, dtype=np.float32)
    Wh = np.asarray(Wh, dtype=np.float32)
    wcat = np.concatenate([
        Wr[:, :I].T, Wz[:, :I].T, Wh[:, :I].T,
        Wr[:, I:].T, Wz[:, I:].T, Wh[:, I:].T,
        -Wr[:, I:].T, -Wz[:, I:].T], axis=1)
    brow = np.concatenate([np.asarray(br, np.float32),
                           np.asarray(bz, np.float32)])[None, :]
    shared = {
        "wcat": np.ascontiguousarray(wcat).astype(BF16NP),
        "brow": np.ascontiguousarray(brow).astype(BF16NP),
        "ones": np.ones((1, C * BC), dtype=BF16NP),
        "bh": np.ascontiguousarray(
            np.asarray(bh, np.float32).reshape(128, 1)),
    }
    in_maps = []
    for c in range(NCORES):
        sl = slice(c * BL, (c + 1) * BL)
        in_maps.append({
            "xT": np.ascontiguousarray(
                x[sl].transpose(1, 2, 0)).astype(BF16NP),
            "aT": np.ascontiguousarray(np.broadcast_to(
                a[sl].T.reshape(1, T * BL), (128, T * BL))).astype(BF16NP),
            **shared,
        })
    return in_maps


def kernel(inputs, attention_scores, Wz, bz, Wr, br, Wh, bh):
    global _compiled
    if _compiled is None:
        _compiled = build_program()
    nc = _compiled
    in_maps = _prep_inputs(inputs, attention_scores, Wz, bz, Wr, br, Wh, bh)
    res = run_bass_kernel_spmd(nc, in_maps, list(range(NCORES)))
    out = np.empty((B, H), dtype=np.float32)
    for c in range(NCORES):
        out[c * BL:(c + 1) * BL, :] = res.results[c]["h_out"].T
    return out

